# revision 18
# baseline (speedup 1.0000x reference)
"""GAT (2-layer, 3-head) forward on 8 Trainium2 NeuronCores.

Sharding: nodes split 8 ways; each core owns 12544 padded destination nodes
and all their incoming edges (1D graph partition per the spec hint). A
channel-major node table (h | a_src | a_dst, 15 ch) is replicated into SBUF
as 4 quarters x 2 copies across the 8 GPSIMD 16-partition groups; per-edge
features stream out via ap_gather with per-group index streams in
dst-canonical order. Destinations are sorted by their max per-quarter
in-degree and packed into 224-dst chunks with a per-chunk slot width K
(capacity 2K per (dst, quarter); A/B copy balancing; rare overflow edges go
to dedicated overflow rows processed as two extra K=8 chunks). Edge softmax
weights run densely over the slot grid (leaky-relu on DVE, exp on ACT);
weighted messages reduce per dst on DVE; per-2-chunk partial tables
[128, 448] stream to DRAM.
The cheap O(N) combine (overflow fold, group sum, softmax normalization,
bias/ELU, W2 projection, head mean + log_softmax) runs on the host between
launches. Three NEFF launches: (A) x @ W1aug table build on PE, then one
shared edge program run twice (layers 1 and 2)."""
import sys
import types

sys.path.insert(0, "/opt/trn_rl_repo")
import numpy as np

N_NODES = 100000
IN_DIM = 256
HID = 3
HEADS = 3
NCLS = 3
NEG = 0.2
EPS = 1e-16

NQ = 4
QREAL = 25000
QN = 25088
NPAD = NQ * QN          # 100352
NCORE = 8
CN = NPAD // NCORE      # 12544
KMAX = 8
DCHUNK = 224
NCHUNK = CN // DCHUNK   # 56
OVN = 2 * DCHUNK        # 448 overflow rows
RPAD = CN + OVN
SENT = QREAL
CH = 15
BIG_NEG = -30000.0
PREF = 3

LAST_STATS = {}


def _install_ntff_hook_module():
    if "antenv.axon_hooks" in sys.modules:
        return
    mod = types.ModuleType("antenv.axon_hooks")
    state = {"hook": None, "tried": False}

    def set_axon_ntff_profile_hook(hook):
        state["hook"] = hook

    def get_axon_ntff_profile_hook():
        if state["hook"] is None and not state["tried"]:
            state["tried"] = True
            try:
                from trn_agent_boot.trn_boot import _ntff_profile_via_ctypes

                state["hook"] = _ntff_profile_via_ctypes("/opt/axon/libaxon_pjrt.so")
            except Exception:
                state["hook"] = None
        return state["hook"]

    mod.set_axon_ntff_profile_hook = set_axon_ntff_profile_hook
    mod.get_axon_ntff_profile_hook = get_axon_ntff_profile_hook
    sys.modules["antenv.axon_hooks"] = mod


_install_ntff_hook_module()

import concourse.bass as bass
import concourse.mybir as mybir
import concourse.tile as tile
from concourse.bass_utils import run_bass_kernel_spmd
from concourse.library_overlay import lower_extended_insts
from concourse import library_config

F32 = mybir.dt.float32
I16 = mybir.dt.int16
ALU = mybir.AluOpType
ACT = mybir.ActivationFunctionType


def _split_wide_waits(nc):
    """Walrus here caps sync-wait commands per instruction; hoist excess waits
    onto preceding same-engine nofuse NOPs (engines execute in order)."""
    for fn in nc.m.functions:
        for bb in fn.blocks:
            new_insts = []
            for inst in bb.instructions:
                keep = 0 if isinstance(inst, mybir.InstDrain) else 1
                si = inst.sync_info
                if si is not None and si.on_wait is not None and len(si.on_wait) > keep:
                    waits = list(si.on_wait)
                    head, rest = (waits[:-keep], waits[-keep:]) if keep else (waits, [])
                    while head:
                        chunk, head = head[:1], head[1:]
                        nop = mybir.InstNoOp(name=f"I-{nc.next_id()}", ins=[], outs=[])
                        nop.engine = inst.engine
                        nop.bass_nofuse = True
                        nop.sync_info = mybir.SyncInfo(on_wait=chunk, on_update=[])
                        nc.register_instruction(nop, overwrite=True)
                        new_insts.append(nop)
                    inst.sync_info = mybir.SyncInfo(
                        on_wait=rest, on_update=list(si.on_update or [])
                    )
                new_insts.append(inst)
            bb.instructions.clear()
            for i in new_insts:
                bb.add_instruction(i)


def _run(nc, in_maps, trace=False):
    lower_extended_insts(nc)
    _split_wide_waits(nc)
    return run_bass_kernel_spmd(nc, in_maps, core_ids=list(range(NCORE)), trace=trace)


# ---------------------------------------------------------------- launch A
def _build_phase_a():
    nc = bass.Bass("TRN2")
    xT_d = nc.dram_tensor("xT", [IN_DIM, CN], F32, kind="ExternalInput")
    w1aug_d = nc.dram_tensor("w1aug", [128, 2 * CH], F32, kind="ExternalInput")
    tab_d = nc.dram_tensor("tab", [CH, CN], F32, kind="ExternalOutput")

    with tile.TileContext(nc) as tc:
        with (
            tc.tile_pool(name="const", bufs=1) as cpool,
            tc.tile_pool(name="io", bufs=4) as iopool,
            tc.tile_pool(name="ps", bufs=4, space="PSUM") as pspool,
        ):
            w1aug = cpool.tile([128, 2 * CH], F32)
            nc.sync.dma_start(w1aug[:], w1aug_d[:])
            for c in range(NCHUNK):
                cols = slice(DCHUNK * c, DCHUNK * (c + 1))
                ps = pspool.tile([CH, DCHUNK], F32, tag="ps")
                for k in range(2):
                    xc = iopool.tile([128, DCHUNK], F32, tag=f"xc{k}")
                    eng = nc.scalar if k == 0 else nc.sync
                    eng.dma_start(xc[:], xT_d[128 * k:128 * (k + 1), cols])
                    nc.tensor.matmul(
                        out=ps[:],
                        lhsT=w1aug[:, CH * k:CH * (k + 1)],
                        rhs=xc[:],
                        start=(k == 0),
                        stop=(k == 1),
                    )
                ob = iopool.tile([CH, DCHUNK], F32, tag="ob")
                nc.vector.tensor_copy(out=ob[:], in_=ps[:])
                nc.scalar.dma_start(tab_d[:, cols], ob[:])
    return nc


# ---------------------------------------------------------------- edge launch
def _build_edge(Ks, scol_off, stot):
    """One edge-layer pass: gather + edge softmax weights + weighted reduce.
    Ks: per-chunk slot width (58 entries, last two are the overflow chunks).
    scol_off: per-chunk offset into the wrapped idx stream (units of 16 idxs).
    stot: total per-group slots (idx stream length)."""
    nc = bass.Bass("TRN2")
    tab_d = nc.dram_tensor("tabf", [CH, NPAD], F32, kind="ExternalInput")
    idx_d = nc.dram_tensor("idxs", [128, stot // 16], I16, kind="ExternalInput")
    adrep_d = nc.dram_tensor("adrep", [24, RPAD], F32, kind="ExternalInput")
    parts_d = nc.dram_tensor("parts", [128, RPAD], F32, kind="ExternalOutput")

    nch = len(Ks)
    SMAX = DCHUNK * KMAX
    with tile.TileContext(nc) as tc:
        with (
            tc.tile_pool(name="big", bufs=1) as bigpool,
            tc.tile_pool(name="gp", bufs=5) as gpool,
            tc.tile_pool(name="wt", bufs=3) as wtpool,
            tc.tile_pool(name="idx", bufs=PREF + 2) as idxpool,
            tc.tile_pool(name="ad", bufs=2) as adpool,
            tc.tile_pool(name="pb", bufs=3) as pbpool,
        ):
            table = bigpool.tile([128, QN], F32)
            for g in range(8):
                q = g % 4
                eng = nc.sync if g % 2 == 0 else nc.scalar
                eng.dma_start(
                    table[16 * g:16 * g + CH, :], tab_d[:, QN * q:QN * (q + 1)]
                )
            w9s = [
                bigpool.tile([128, SMAX], F32, name=f"w9_{i}") for i in range(3)
            ]
            for t in w9s:
                nc.vector.memset(t[:], 1.0)

            tab_in = table[:].rearrange("p (n d) -> p n d", d=1)
            nc.gpsimd.load_library(library_config.ap_gather)

            def load_idxc(c):
                w = DCHUNK * Ks[c] // 16
                t = idxpool.tile([128, SMAX // 16], I16, tag="idxc")
                nc.scalar.dma_start(
                    t[:, :w], idx_d[:, scol_off[c]:scol_off[c] + w]
                )
                return t

            def load_a24(c):
                t = adpool.tile([24, 2 * DCHUNK], F32, tag="a24")
                nc.scalar.dma_start(
                    t[:], adrep_d[:, DCHUNK * c:DCHUNK * (c + 2)]
                )
                return t

            idxq = {c: load_idxc(c) for c in range(min(PREF, nch))}
            a24q = {c: load_a24(c) for c in range(0, min(PREF + 1, nch), 2)}
            pbq = {}

            for c in range(nch):
                K = Ks[c]
                S = DCHUNK * K
                if c + PREF < nch:
                    idxq[c + PREF] = load_idxc(c + PREF)
                    if (c + PREF) % 2 == 0:
                        a24q[c + PREF] = load_a24(c + PREF)
                idxc = idxq.pop(c)
                g_t = gpool.tile([128, SMAX], F32, tag="g")
                nc.gpsimd.ap_gather(
                    out_ap=g_t[:, :S].rearrange("p (n d) -> p n d", d=1),
                    in_ap=tab_in,
                    idxs_ap=idxc[:, :S // 16],
                    channels=128,
                    num_elems=QN,
                    d=1,
                    num_idxs=S,
                )
                a24 = a24q[c - (c % 2)]
                ahalf = a24[:, (c % 2) * DCHUNK:(c % 2 + 1) * DCHUNK]
                wt = wtpool.tile([24, SMAX], F32, tag="wt")
                for g in range(8):
                    nc.sync.dma_start(
                        wt[3 * g:3 * g + 3, :S], g_t[16 * g + 9:16 * g + 12, :S]
                    )
                wt3 = wt[:, :S].rearrange("p (n j) -> p n j", j=K)
                nc.vector.tensor_tensor(
                    out=wt3, in0=wt3,
                    in1=ahalf.to_broadcast([24, DCHUNK, K]), op=ALU.add,
                )
                nc.vector.scalar_tensor_tensor(
                    out=wt[:, :S], in0=wt[:, :S], scalar=NEG, in1=wt[:, :S],
                    op0=ALU.mult, op1=ALU.max,
                )
                nc.scalar.activation(out=wt[:, :S], in_=wt[:, :S], func=ACT.Exp)
                w9 = w9s[c % 3]
                for h in range(3):
                    nc.sync.dma_start(g_t[9 + h::16, :S], wt[h::3, :S])
                    for ch3 in range(3):
                        nc.sync.dma_start(w9[3 * h + ch3::16, :S], wt[h::3, :S])
                nc.vector.tensor_tensor(
                    out=g_t[:, :S], in0=g_t[:, :S], in1=w9[:, :S], op=ALU.mult
                )
                if c % 2 == 0:
                    pbq[c // 2] = pbpool.tile([128, 2 * DCHUNK], F32, name="pb", tag="pb")
                pb = pbq[c // 2]
                nc.vector.tensor_reduce(
                    out=pb[:, (c % 2) * DCHUNK:(c % 2 + 1) * DCHUNK],
                    in_=g_t[:, :S].rearrange("p (n j) -> p n j", j=K),
                    axis=mybir.AxisListType.X,
                    op=ALU.add,
                )
                if c % 2 == 1:
                    b = c // 2
                    nc.scalar.dma_start(
                        parts_d[:, 2 * DCHUNK * b:2 * DCHUNK * (b + 1)],
                        pbq.pop(b)[:],
                    )
    return nc


# ---------------------------------------------------------------- host side
def _relabel(n):
    q = n // QREAL
    return q * QN + n % QREAL


def _wrap16(stream):
    """[8, S] group streams -> [128, S//16] ap_gather idx layout."""
    g, s = stream.shape
    w = stream.reshape(g, s // 16, 16).transpose(0, 2, 1)
    return np.ascontiguousarray(w.reshape(g * 16, s // 16))


def _pack_edges(src, dst):
    srcN = _relabel(src.astype(np.int64))
    dstN = _relabel(dst.astype(np.int64))
    core = dstN // CN
    dloc = dstN % CN
    q = srcN // QN
    sloc = (srcN % QN).astype(np.int16)

    cnt = np.zeros((NCORE, CN, 4), np.int64)
    np.add.at(cnt, (core, dloc, q), 1)
    maxq = cnt.max(axis=2)

    orders = []
    Ks_all = []
    for c in range(NCORE):
        order = np.argsort(-maxq[c], kind="stable")
        m = maxq[c][order]
        Ks = []
        for b in range(NCHUNK):
            mm = m[b * DCHUNK:(b + 1) * DCHUNK].max()
            Ks.append(min(KMAX, max(1, int(-(-int(mm) // 2)))))
        orders.append(order)
        Ks_all.append(Ks + [KMAX, KMAX])
    # shared chunk schedule across cores (program is shared): use per-chunk max
    Ks = [max(Ks_all[c][i] for c in range(NCORE)) for i in range(NCHUNK + 2)]
    # sorted position of each dst
    spos = np.empty((NCORE, CN), np.int64)
    for c in range(NCORE):
        spos[c][orders[c]] = np.arange(CN)

    karr = np.array(Ks[:NCHUNK], np.int64)
    base = np.concatenate([[0], np.cumsum(DCHUNK * karr)])  # slot base per chunk
    stot = int(base[-1]) + OVN * KMAX
    ovbase = int(base[-1])

    # slot offset for each sorted dst position
    pos_chunk = np.arange(CN) // DCHUNK
    slot0 = base[pos_chunk] + (np.arange(CN) % DCHUNK) * karr[pos_chunk]
    cap = 2 * karr[pos_chunk]  # capacity per (dst, q) at sorted position

    key = (core * CN + dloc) * 4 + q
    order = np.argsort(key, kind="stable")
    ks = key[order]
    grp_start = np.r_[0, np.flatnonzero(np.diff(ks)) + 1]
    sizes = np.diff(np.r_[grp_start, len(ks)])
    rank = np.arange(len(ks)) - np.repeat(grp_start, sizes)

    co, dl, qo, sl = core[order], dloc[order], q[order], sloc[order]
    sp = spos[co, dl]
    scap = cap[sp]
    ssl0 = slot0[sp]
    kk = karr[sp // DCHUNK]

    streams = np.full((NCORE, 8, stot), SENT, dtype=np.int16)
    ovidx = np.full((NCORE, CN), OVN - 1, dtype=np.int16)
    ovdst = [[] for _ in range(NCORE)]

    main = rank < scap
    gmain = qo[main] + 4 * (rank[main] & 1)
    pos = ssl0[main] + (rank[main] >> 1)
    streams[co[main], gmain, pos] = sl[main]

    for i in np.flatnonzero(~main):
        c, s_p, qq, s_, r = co[i], int(sp[i]), qo[i], sl[i], rank[i]
        if ovidx[c, s_p] == OVN - 1:
            row = len(ovdst[c])
            assert row < OVN - 1, "overflow area exhausted"
            ovidx[c, s_p] = row
            ovdst[c].append(s_p)
        rr = r - scap[i]
        assert rr < 16, "overflow capacity exceeded"
        g = qq + 4 * (rr & 1)
        streams[c, g, ovbase + int(ovidx[c, s_p]) * KMAX + (rr >> 1)] = s_
    return streams, ovidx, ovdst, orders, Ks, base, stot


def kernel(x, edge_index, W1, att_src1, att_dst1, b1, W2, att_src2, att_dst2, b2):
    import os as _os
    import time as _time

    x = np.asarray(x, np.float32)
    W1 = np.asarray(W1, np.float32)
    W2 = np.asarray(W2, np.float32)
    b1v = np.asarray(b1, np.float32)
    b2v = np.asarray(b2, np.float32)

    loops = np.arange(N_NODES, dtype=np.int64)
    src = np.concatenate([np.asarray(edge_index[0], np.int64), loops])
    dst = np.concatenate([np.asarray(edge_index[1], np.int64), loops])
    streams, ovidx, ovdst, orders, Ks, base, stot = _pack_edges(src, dst)

    xP = np.zeros((NPAD, IN_DIM), np.float32)
    xP[_relabel(np.arange(N_NODES))] = x
    xT = np.ascontiguousarray(xP.T)

    def attw(att_s, att_d):
        a = np.zeros((HEADS * HID, 6), np.float32)
        for h in range(HEADS):
            for cc in range(3):
                a[3 * h + cc, h] = np.asarray(att_s, np.float32)[h, cc]
                a[3 * h + cc, 3 + h] = np.asarray(att_d, np.float32)[h, cc]
        return a

    attw1 = attw(att_src1, att_dst1)
    attw2 = attw(att_src2, att_dst2)

    w1aug = np.zeros((128, 2 * CH), np.float32)
    v1 = W1 @ attw1
    for k in range(2):
        w1aug[:, CH * k:CH * k + 9] = W1[128 * k:128 * (k + 1), :]
        w1aug[:, CH * k + 9:CH * k + 15] = v1[128 * k:128 * (k + 1), :]
    w2aug = np.concatenate([W2, W2 @ attw2], axis=1)  # [9, 15]

    # wrapped idx stream: per-chunk 16-wrap, concatenated
    kall = Ks[:NCHUNK] + [KMAX, KMAX]
    scol_off = []
    off = 0
    for K in kall:
        scol_off.append(off)
        off += DCHUNK * K // 16
    idx_wr = []
    for c in range(NCORE):
        blocks = []
        for i, K in enumerate(kall):
            lo = int(base[i]) if i < NCHUNK else int(base[-1]) + (i - NCHUNK) * DCHUNK * KMAX
            blocks.append(_wrap16(streams[c][:, lo:lo + DCHUNK * K]))
        idx_wr.append(np.concatenate(blocks, axis=1))

    def make_adrep(tab):
        out = []
        for c in range(NCORE):
            srt = orders[c]
            ad = np.zeros((3, RPAD), np.float32)
            ad[:, :CN] = tab[12:15, CN * c + srt]
            for i, d in enumerate(ovdst[c]):
                ad[:, CN + i] = tab[12:15, CN * c + srt[d]]
            rep = np.zeros((24, RPAD), np.float32)
            for g in range(8):
                rep[3 * g:3 * g + 3, :] = ad
            out.append(rep)
        return out

    padmask = np.zeros(NPAD, bool)
    for qq in range(NQ):
        padmask[QN * qq + QREAL:QN * (qq + 1)] = True

    def combine(parts_list):
        """Fold overflow + group-sum -> per-core num[9, CN], den[3, CN]
        in sorted dst order."""
        nums, dens = [], []
        for c in range(NCORE):
            P = parts_list[c].reshape(8, 16, RPAD)
            num = P[:, 0:9, :].sum(axis=0)
            den = P[:, 9:12, :].sum(axis=0)
            ovx = ovidx[c]
            num[:, :CN] += num[:, CN:][:, ovx]
            den[:, :CN] += den[:, CN:][:, ovx]
            nums.append(num[:, :CN])
            dens.append(den[:, :CN])
        return nums, dens

    trace = bool(int(_os.environ.get("KERNEL_TRACE", "0")))
    stats = {}
    t0 = _time.time()

    ncA = _build_phase_a()
    in_maps = [
        {
            "xT": np.ascontiguousarray(xT[:, CN * c:CN * (c + 1)]),
            "w1aug": w1aug,
        }
        for c in range(NCORE)
    ]
    resA = _run(ncA, in_maps, trace=trace)
    stats["A_ns"] = resA.exec_time_ns
    tab1 = np.concatenate([resA.results[c]["tab"] for c in range(NCORE)], axis=1)
    tab1[9:12, padmask] = BIG_NEG

    ncE = _build_edge(kall, scol_off, stot)

    def run_edge(tab, tag):
        adreps = make_adrep(tab)
        in_maps = [
            {"tabf": tab, "idxs": idx_wr[c], "adrep": adreps[c]}
            for c in range(NCORE)
        ]
        res = _run(ncE, in_maps, trace=trace)
        stats[tag] = res.exec_time_ns
        return [res.results[c]["parts"] for c in range(NCORE)]

    nums, dens = combine(run_edge(tab1, "B_ns"))
    tab2 = np.full((CH, NPAD), 0.0, np.float32)
    for c in range(NCORE):
        hag = nums[c] / (dens[c].repeat(3, axis=0) + EPS) + b1v[:, None]
        v = np.maximum(hag, 0) + np.exp(np.minimum(hag, 0)) - 1.0
        t2 = w2aug.T @ v  # [15, CN] sorted order
        tab2[:, CN * c + orders[c]] = t2
    tab2[9:12, padmask] = BIG_NEG

    nums, dens = combine(run_edge(tab2, "C_ns"))
    out = np.zeros((N_NODES, NCLS), np.float32)
    outP = np.zeros((NPAD, NCLS), np.float32)
    for c in range(NCORE):
        hag = nums[c] / (dens[c].repeat(3, axis=0) + EPS)
        z = hag.reshape(3, 3, CN).mean(axis=0) + b2v[:, None]  # [3, CN]
        z = z - np.log(np.exp(z).sum(axis=0, keepdims=True))
        outP[CN * c + orders[c]] = z.T
    out = outP[_relabel(np.arange(N_NODES))]
    stats["wall_s"] = _time.time() - t0

    LAST_STATS.clear()
    LAST_STATS.update(stats)
    return np.ascontiguousarray(out, dtype=np.float32)


# revision 19
# speedup vs baseline: 1.0167x; 1.0167x over previous
"""GAT (2-layer, 3-head) forward on 8 Trainium2 NeuronCores.

Sharding: nodes split 8 ways; each core owns 12544 padded destination nodes
and all their incoming edges (1D graph partition per the spec hint). A
channel-major node table (h | a_src | a_dst, 15 ch) is replicated into SBUF
as 4 quarters x 2 copies across the 8 GPSIMD 16-partition groups; per-edge
features stream out via ap_gather with per-group index streams in
dst-canonical order. Destinations are sorted by their max per-quarter
in-degree and packed into 224-dst chunks with a per-chunk slot width K
(capacity 2K per (dst, quarter); A/B copy balancing; rare overflow edges go
to dedicated overflow rows processed as two extra K=8 chunks). Edge softmax
weights run densely over the slot grid (leaky-relu on DVE, exp on ACT);
weighted messages reduce per dst on DVE; per-2-chunk partial tables
[128, 448] stream to DRAM.
The cheap O(N) combine (overflow fold, group sum, softmax normalization,
bias/ELU, W2 projection, head mean + log_softmax) runs on the host between
launches. Three NEFF launches: (A) x @ W1aug table build on PE, then one
shared edge program run twice (layers 1 and 2)."""
import sys
import types

sys.path.insert(0, "/opt/trn_rl_repo")
import numpy as np

N_NODES = 100000
IN_DIM = 256
HID = 3
HEADS = 3
NCLS = 3
NEG = 0.2
EPS = 1e-16

NQ = 4
QREAL = 25000
QN = 25088
NPAD = NQ * QN          # 100352
NCORE = 8
CN = NPAD // NCORE      # 12544
KMAX = 8
DCHUNK = 224
NCHUNK = CN // DCHUNK   # 56
OVN = 2 * DCHUNK        # 448 overflow rows
RPAD = CN + OVN
SENT = QREAL
CH = 15
BIG_NEG = -30000.0
PREF = 3
KOV = 6

LAST_STATS = {}


def _install_ntff_hook_module():
    if "antenv.axon_hooks" in sys.modules:
        return
    mod = types.ModuleType("antenv.axon_hooks")
    state = {"hook": None, "tried": False}

    def set_axon_ntff_profile_hook(hook):
        state["hook"] = hook

    def get_axon_ntff_profile_hook():
        if state["hook"] is None and not state["tried"]:
            state["tried"] = True
            try:
                from trn_agent_boot.trn_boot import _ntff_profile_via_ctypes

                state["hook"] = _ntff_profile_via_ctypes("/opt/axon/libaxon_pjrt.so")
            except Exception:
                state["hook"] = None
        return state["hook"]

    mod.set_axon_ntff_profile_hook = set_axon_ntff_profile_hook
    mod.get_axon_ntff_profile_hook = get_axon_ntff_profile_hook
    sys.modules["antenv.axon_hooks"] = mod


_install_ntff_hook_module()

import concourse.bass as bass
import concourse.mybir as mybir
import concourse.tile as tile
from concourse.bass_utils import run_bass_kernel_spmd
from concourse.library_overlay import lower_extended_insts
from concourse import library_config

F32 = mybir.dt.float32
I16 = mybir.dt.int16
ALU = mybir.AluOpType
ACT = mybir.ActivationFunctionType


def _split_wide_waits(nc):
    """Walrus here caps sync-wait commands per instruction; hoist excess waits
    onto preceding same-engine nofuse NOPs (engines execute in order)."""
    for fn in nc.m.functions:
        for bb in fn.blocks:
            new_insts = []
            for inst in bb.instructions:
                keep = 0 if isinstance(inst, mybir.InstDrain) else 1
                si = inst.sync_info
                if si is not None and si.on_wait is not None and len(si.on_wait) > keep:
                    waits = list(si.on_wait)
                    head, rest = (waits[:-keep], waits[-keep:]) if keep else (waits, [])
                    while head:
                        chunk, head = head[:1], head[1:]
                        nop = mybir.InstNoOp(name=f"I-{nc.next_id()}", ins=[], outs=[])
                        nop.engine = inst.engine
                        nop.bass_nofuse = True
                        nop.sync_info = mybir.SyncInfo(on_wait=chunk, on_update=[])
                        nc.register_instruction(nop, overwrite=True)
                        new_insts.append(nop)
                    inst.sync_info = mybir.SyncInfo(
                        on_wait=rest, on_update=list(si.on_update or [])
                    )
                new_insts.append(inst)
            bb.instructions.clear()
            for i in new_insts:
                bb.add_instruction(i)


def _run(nc, in_maps, trace=False):
    lower_extended_insts(nc)
    _split_wide_waits(nc)
    return run_bass_kernel_spmd(nc, in_maps, core_ids=list(range(NCORE)), trace=trace)


# ---------------------------------------------------------------- launch A
def _build_phase_a():
    nc = bass.Bass("TRN2")
    xT_d = nc.dram_tensor("xT", [IN_DIM, CN], F32, kind="ExternalInput")
    w1aug_d = nc.dram_tensor("w1aug", [128, 2 * CH], F32, kind="ExternalInput")
    tab_d = nc.dram_tensor("tab", [CH, CN], F32, kind="ExternalOutput")

    with tile.TileContext(nc) as tc:
        with (
            tc.tile_pool(name="const", bufs=1) as cpool,
            tc.tile_pool(name="io", bufs=4) as iopool,
            tc.tile_pool(name="ps", bufs=4, space="PSUM") as pspool,
        ):
            w1aug = cpool.tile([128, 2 * CH], F32)
            nc.sync.dma_start(w1aug[:], w1aug_d[:])
            for c in range(NCHUNK):
                cols = slice(DCHUNK * c, DCHUNK * (c + 1))
                ps = pspool.tile([CH, DCHUNK], F32, tag="ps")
                for k in range(2):
                    xc = iopool.tile([128, DCHUNK], F32, tag=f"xc{k}")
                    eng = nc.scalar if k == 0 else nc.sync
                    eng.dma_start(xc[:], xT_d[128 * k:128 * (k + 1), cols])
                    nc.tensor.matmul(
                        out=ps[:],
                        lhsT=w1aug[:, CH * k:CH * (k + 1)],
                        rhs=xc[:],
                        start=(k == 0),
                        stop=(k == 1),
                    )
                ob = iopool.tile([CH, DCHUNK], F32, tag="ob")
                nc.vector.tensor_copy(out=ob[:], in_=ps[:])
                nc.scalar.dma_start(tab_d[:, cols], ob[:])
    return nc


# ---------------------------------------------------------------- edge launch
def _build_edge(Ks, scol_off, stot):
    """One edge-layer pass: gather + edge softmax weights + weighted reduce.
    Ks: per-chunk slot width (58 entries, last two are the overflow chunks).
    scol_off: per-chunk offset into the wrapped idx stream (units of 16 idxs).
    stot: total per-group slots (idx stream length)."""
    nc = bass.Bass("TRN2")
    tab_d = nc.dram_tensor("tabf", [CH, NPAD], F32, kind="ExternalInput")
    idx_d = nc.dram_tensor("idxs", [128, stot // 16], I16, kind="ExternalInput")
    adrep_d = nc.dram_tensor("adrep", [24, RPAD], F32, kind="ExternalInput")
    parts_d = nc.dram_tensor("parts", [128, RPAD], F32, kind="ExternalOutput")

    nch = len(Ks)
    SMAX = DCHUNK * KMAX
    with tile.TileContext(nc) as tc:
        with (
            tc.tile_pool(name="big", bufs=1) as bigpool,
            tc.tile_pool(name="gp", bufs=5) as gpool,
            tc.tile_pool(name="wt", bufs=3) as wtpool,
            tc.tile_pool(name="idx", bufs=PREF + 2) as idxpool,
            tc.tile_pool(name="ad", bufs=2) as adpool,
            tc.tile_pool(name="pb", bufs=3) as pbpool,
        ):
            table = bigpool.tile([128, QN], F32)
            for g in range(8):
                q = g % 4
                eng = nc.sync if g % 2 == 0 else nc.scalar
                eng.dma_start(
                    table[16 * g:16 * g + CH, :], tab_d[:, QN * q:QN * (q + 1)]
                )
            w9s = [
                bigpool.tile([128, SMAX], F32, name=f"w9_{i}") for i in range(3)
            ]
            for t in w9s:
                nc.vector.memset(t[:], 1.0)

            tab_in = table[:].rearrange("p (n d) -> p n d", d=1)
            nc.gpsimd.load_library(library_config.ap_gather)

            def load_idxc(c):
                w = DCHUNK * Ks[c] // 16
                t = idxpool.tile([128, SMAX // 16], I16, tag="idxc")
                nc.scalar.dma_start(
                    t[:, :w], idx_d[:, scol_off[c]:scol_off[c] + w]
                )
                return t

            def load_a24(c):
                t = adpool.tile([24, 2 * DCHUNK], F32, tag="a24")
                nc.scalar.dma_start(
                    t[:], adrep_d[:, DCHUNK * c:DCHUNK * (c + 2)]
                )
                return t

            idxq = {c: load_idxc(c) for c in range(min(PREF, nch))}
            a24q = {c: load_a24(c) for c in range(0, min(PREF + 1, nch), 2)}
            pbq = {}

            for c in range(nch):
                K = Ks[c]
                S = DCHUNK * K
                if c + PREF < nch:
                    idxq[c + PREF] = load_idxc(c + PREF)
                    if (c + PREF) % 2 == 0:
                        a24q[c + PREF] = load_a24(c + PREF)
                idxc = idxq.pop(c)
                g_t = gpool.tile([128, SMAX], F32, tag="g")
                nc.gpsimd.ap_gather(
                    out_ap=g_t[:, :S].rearrange("p (n d) -> p n d", d=1),
                    in_ap=tab_in,
                    idxs_ap=idxc[:, :S // 16],
                    channels=128,
                    num_elems=QN,
                    d=1,
                    num_idxs=S,
                )
                a24 = a24q[c - (c % 2)]
                ahalf = a24[:, (c % 2) * DCHUNK:(c % 2 + 1) * DCHUNK]
                wt = wtpool.tile([24, SMAX], F32, tag="wt")
                for g in range(8):
                    nc.sync.dma_start(
                        wt[3 * g:3 * g + 3, :S], g_t[16 * g + 9:16 * g + 12, :S]
                    )
                wt3 = wt[:, :S].rearrange("p (n j) -> p n j", j=K)
                nc.vector.tensor_tensor(
                    out=wt3, in0=wt3,
                    in1=ahalf.to_broadcast([24, DCHUNK, K]), op=ALU.add,
                )
                nc.vector.scalar_tensor_tensor(
                    out=wt[:, :S], in0=wt[:, :S], scalar=NEG, in1=wt[:, :S],
                    op0=ALU.mult, op1=ALU.max,
                )
                nc.scalar.activation(out=wt[:, :S], in_=wt[:, :S], func=ACT.Exp)
                w9 = w9s[c % 3]
                for h in range(3):
                    nc.sync.dma_start(g_t[9 + h::16, :S], wt[h::3, :S])
                    for ch3 in range(3):
                        nc.sync.dma_start(w9[3 * h + ch3::16, :S], wt[h::3, :S])
                nc.vector.tensor_tensor(
                    out=g_t[:, :S], in0=g_t[:, :S], in1=w9[:, :S], op=ALU.mult
                )
                if c % 2 == 0:
                    pbq[c // 2] = pbpool.tile([128, 2 * DCHUNK], F32, name="pb", tag="pb")
                pb = pbq[c // 2]
                nc.vector.tensor_reduce(
                    out=pb[:, (c % 2) * DCHUNK:(c % 2 + 1) * DCHUNK],
                    in_=g_t[:, :S].rearrange("p (n j) -> p n j", j=K),
                    axis=mybir.AxisListType.X,
                    op=ALU.add,
                )
                if c % 2 == 1:
                    b = c // 2
                    nc.scalar.dma_start(
                        parts_d[:, 2 * DCHUNK * b:2 * DCHUNK * (b + 1)],
                        pbq.pop(b)[:],
                    )
    return nc


# ---------------------------------------------------------------- host side
def _relabel(n):
    q = n // QREAL
    return q * QN + n % QREAL


def _wrap16(stream):
    """[8, S] group streams -> [128, S//16] ap_gather idx layout."""
    g, s = stream.shape
    w = stream.reshape(g, s // 16, 16).transpose(0, 2, 1)
    return np.ascontiguousarray(w.reshape(g * 16, s // 16))


def _pack_edges(src, dst):
    srcN = _relabel(src.astype(np.int64))
    dstN = _relabel(dst.astype(np.int64))
    core = dstN // CN
    dloc = dstN % CN
    q = srcN // QN
    sloc = (srcN % QN).astype(np.int16)

    cnt = np.zeros((NCORE, CN, 4), np.int64)
    np.add.at(cnt, (core, dloc, q), 1)
    maxq = cnt.max(axis=2)

    orders = []
    Ks_all = []
    for c in range(NCORE):
        order = np.argsort(-maxq[c], kind="stable")
        m = maxq[c][order]
        Ks = []
        for b in range(NCHUNK):
            mm = m[b * DCHUNK:(b + 1) * DCHUNK].max()
            Ks.append(min(KMAX, max(1, int(-(-int(mm) // 2)))))
        orders.append(order)
        Ks_all.append(Ks + [KOV, KOV])
    # shared chunk schedule across cores (program is shared): use per-chunk max
    Ks = [max(Ks_all[c][i] for c in range(NCORE)) for i in range(NCHUNK + 2)]
    # sorted position of each dst
    spos = np.empty((NCORE, CN), np.int64)
    for c in range(NCORE):
        spos[c][orders[c]] = np.arange(CN)

    karr = np.array(Ks[:NCHUNK], np.int64)
    base = np.concatenate([[0], np.cumsum(DCHUNK * karr)])  # slot base per chunk
    stot = int(base[-1]) + OVN * KOV
    ovbase = int(base[-1])

    # slot offset for each sorted dst position
    pos_chunk = np.arange(CN) // DCHUNK
    slot0 = base[pos_chunk] + (np.arange(CN) % DCHUNK) * karr[pos_chunk]
    cap = 2 * karr[pos_chunk]  # capacity per (dst, q) at sorted position

    key = (core * CN + dloc) * 4 + q
    order = np.argsort(key, kind="stable")
    ks = key[order]
    grp_start = np.r_[0, np.flatnonzero(np.diff(ks)) + 1]
    sizes = np.diff(np.r_[grp_start, len(ks)])
    rank = np.arange(len(ks)) - np.repeat(grp_start, sizes)

    co, dl, qo, sl = core[order], dloc[order], q[order], sloc[order]
    sp = spos[co, dl]
    scap = cap[sp]
    ssl0 = slot0[sp]
    kk = karr[sp // DCHUNK]

    streams = np.full((NCORE, 8, stot), SENT, dtype=np.int16)
    ovidx = np.full((NCORE, CN), OVN - 1, dtype=np.int16)
    ovdst = [[] for _ in range(NCORE)]

    main = rank < scap
    gmain = qo[main] + 4 * (rank[main] & 1)
    pos = ssl0[main] + (rank[main] >> 1)
    streams[co[main], gmain, pos] = sl[main]

    for i in np.flatnonzero(~main):
        c, s_p, qq, s_, r = co[i], int(sp[i]), qo[i], sl[i], rank[i]
        if ovidx[c, s_p] == OVN - 1:
            row = len(ovdst[c])
            assert row < OVN - 1, "overflow area exhausted"
            ovidx[c, s_p] = row
            ovdst[c].append(s_p)
        rr = r - scap[i]
        assert rr < 2 * KOV, "overflow capacity exceeded"
        g = qq + 4 * (rr & 1)
        streams[c, g, ovbase + int(ovidx[c, s_p]) * KOV + (rr >> 1)] = s_
    return streams, ovidx, ovdst, orders, Ks, base, stot


def kernel(x, edge_index, W1, att_src1, att_dst1, b1, W2, att_src2, att_dst2, b2):
    import os as _os
    import time as _time

    x = np.asarray(x, np.float32)
    W1 = np.asarray(W1, np.float32)
    W2 = np.asarray(W2, np.float32)
    b1v = np.asarray(b1, np.float32)
    b2v = np.asarray(b2, np.float32)

    loops = np.arange(N_NODES, dtype=np.int64)
    src = np.concatenate([np.asarray(edge_index[0], np.int64), loops])
    dst = np.concatenate([np.asarray(edge_index[1], np.int64), loops])
    streams, ovidx, ovdst, orders, Ks, base, stot = _pack_edges(src, dst)

    xP = np.zeros((NPAD, IN_DIM), np.float32)
    xP[_relabel(np.arange(N_NODES))] = x
    xT = np.ascontiguousarray(xP.T)

    def attw(att_s, att_d):
        a = np.zeros((HEADS * HID, 6), np.float32)
        for h in range(HEADS):
            for cc in range(3):
                a[3 * h + cc, h] = np.asarray(att_s, np.float32)[h, cc]
                a[3 * h + cc, 3 + h] = np.asarray(att_d, np.float32)[h, cc]
        return a

    attw1 = attw(att_src1, att_dst1)
    attw2 = attw(att_src2, att_dst2)

    w1aug = np.zeros((128, 2 * CH), np.float32)
    v1 = W1 @ attw1
    for k in range(2):
        w1aug[:, CH * k:CH * k + 9] = W1[128 * k:128 * (k + 1), :]
        w1aug[:, CH * k + 9:CH * k + 15] = v1[128 * k:128 * (k + 1), :]
    w2aug = np.concatenate([W2, W2 @ attw2], axis=1)  # [9, 15]

    # wrapped idx stream: per-chunk 16-wrap, concatenated
    kall = Ks[:NCHUNK] + [KOV, KOV]
    scol_off = []
    off = 0
    for K in kall:
        scol_off.append(off)
        off += DCHUNK * K // 16
    idx_wr = []
    for c in range(NCORE):
        blocks = []
        for i, K in enumerate(kall):
            lo = int(base[i]) if i < NCHUNK else int(base[-1]) + (i - NCHUNK) * DCHUNK * KOV
            blocks.append(_wrap16(streams[c][:, lo:lo + DCHUNK * K]))
        idx_wr.append(np.concatenate(blocks, axis=1))

    def make_adrep(tab):
        out = []
        for c in range(NCORE):
            srt = orders[c]
            ad = np.zeros((3, RPAD), np.float32)
            ad[:, :CN] = tab[12:15, CN * c + srt]
            for i, d in enumerate(ovdst[c]):
                ad[:, CN + i] = tab[12:15, CN * c + srt[d]]
            rep = np.zeros((24, RPAD), np.float32)
            for g in range(8):
                rep[3 * g:3 * g + 3, :] = ad
            out.append(rep)
        return out

    padmask = np.zeros(NPAD, bool)
    for qq in range(NQ):
        padmask[QN * qq + QREAL:QN * (qq + 1)] = True

    def combine(parts_list):
        """Fold overflow + group-sum -> per-core num[9, CN], den[3, CN]
        in sorted dst order."""
        nums, dens = [], []
        for c in range(NCORE):
            P = parts_list[c].reshape(8, 16, RPAD)
            num = P[:, 0:9, :].sum(axis=0)
            den = P[:, 9:12, :].sum(axis=0)
            ovx = ovidx[c]
            num[:, :CN] += num[:, CN:][:, ovx]
            den[:, :CN] += den[:, CN:][:, ovx]
            nums.append(num[:, :CN])
            dens.append(den[:, :CN])
        return nums, dens

    trace = bool(int(_os.environ.get("KERNEL_TRACE", "0")))
    stats = {}
    t0 = _time.time()

    ncA = _build_phase_a()
    in_maps = [
        {
            "xT": np.ascontiguousarray(xT[:, CN * c:CN * (c + 1)]),
            "w1aug": w1aug,
        }
        for c in range(NCORE)
    ]
    resA = _run(ncA, in_maps, trace=trace)
    stats["A_ns"] = resA.exec_time_ns
    tab1 = np.concatenate([resA.results[c]["tab"] for c in range(NCORE)], axis=1)
    tab1[9:12, padmask] = BIG_NEG

    ncE = _build_edge(kall, scol_off, stot)

    def run_edge(tab, tag):
        adreps = make_adrep(tab)
        in_maps = [
            {"tabf": tab, "idxs": idx_wr[c], "adrep": adreps[c]}
            for c in range(NCORE)
        ]
        res = _run(ncE, in_maps, trace=trace)
        stats[tag] = res.exec_time_ns
        return [res.results[c]["parts"] for c in range(NCORE)]

    nums, dens = combine(run_edge(tab1, "B_ns"))
    tab2 = np.full((CH, NPAD), 0.0, np.float32)
    for c in range(NCORE):
        hag = nums[c] / (dens[c].repeat(3, axis=0) + EPS) + b1v[:, None]
        v = np.maximum(hag, 0) + np.exp(np.minimum(hag, 0)) - 1.0
        t2 = w2aug.T @ v  # [15, CN] sorted order
        tab2[:, CN * c + orders[c]] = t2
    tab2[9:12, padmask] = BIG_NEG

    nums, dens = combine(run_edge(tab2, "C_ns"))
    out = np.zeros((N_NODES, NCLS), np.float32)
    outP = np.zeros((NPAD, NCLS), np.float32)
    for c in range(NCORE):
        hag = nums[c] / (dens[c].repeat(3, axis=0) + EPS)
        z = hag.reshape(3, 3, CN).mean(axis=0) + b2v[:, None]  # [3, CN]
        z = z - np.log(np.exp(z).sum(axis=0, keepdims=True))
        outP[CN * c + orders[c]] = z.T
    out = outP[_relabel(np.arange(N_NODES))]
    stats["wall_s"] = _time.time() - t0

    LAST_STATS.clear()
    LAST_STATS.update(stats)
    return np.ascontiguousarray(out, dtype=np.float32)
